# revision 19
# baseline (speedup 1.0000x reference)
"""Cross multi-head attention on 8 Trainium2 NeuronCores.

Sharding: pure data-parallel. Core c handles batch b = c//2, query-half
h = c%2 (1024 of 2048 query rows). K/V are recomputed per seq-half (25%
extra PE work) which avoids all collectives.

Per-core kernel (all in T-major layout so no on-chip transposes at all;
the host pre-transposes x/y and pre-packs the weights):
  QT[hp]  [128,Sl] = Wq2[hp].T @ xT      (head-pair packed: rows 0:64 head a,
  KT[hp]  [128,T]  = Wk2[hp].T @ yT       rows 64:128 head b)
  V       [t,c]    = yT.T @ Wv_cat       (natural layout, c = h*64+d)
  scoresT [t,q]    = KT_h.T @ QT_h       (K=64 contraction, two heads run
                                          concurrently via PE row tiling)
  expT    = exp(scoresT * 0.125)         (ACT reads PSUM pair tile directly)
  oT      += V_h.T @ expT                (col-tiled: head a -> psum rows 0:64,
                                          head b -> rows 64:128)
  sums    += ones.T @ expT               (softmax denominator, in psum)
  oT_norm = oT * bcast(1/sums)           (normalization deferred to one
                                          DVE pass, fused with psum->sbuf copy)
  out     = concatT.T @ Wo + bo
Matmul inputs bf16 (fp32 PSUM accumulation), softmax in fp32.
"""

import numpy as np

B, S, T, E, H, D = 4, 2048, 2048, 1024, 16, 64
N_CORES = 8

_compiled = {}


def _dt():
    from concourse import mybir

    return mybir.dt


def build_program(n_hp=8, s_loc=1024, t_len=2048, n_et=8, debug_taps=False):
    """Emit the per-core bass program. Sizes parameterizable for sim tests.

    n_hp: head pairs (heads = 2*n_hp), s_loc: query rows on this core,
    t_len: key rows, n_et: contraction tiles (emb dim = 128*n_et).
    """
    import concourse.tile as tile
    from concourse import bacc

    dt = _dt()
    bf16 = dt.bfloat16
    f32 = dt.float32

    e_dim = 128 * n_et
    c_dim = 128 * n_hp  # concat dim on this core's heads
    n_tt = t_len // 128  # key tiles
    qch = min(512, s_loc)  # query chunk width
    tch = min(512, t_len)
    ech = min(512, e_dim)
    vch = min(512, c_dim)
    n_qc = s_loc // qch  # query chunks for attention
    n_st = s_loc // 128  # output row tiles
    n_ec = e_dim // ech  # output col chunks

    nc = bacc.Bacc("TRN2", target_bir_lowering=False, debug=False)

    # ---- DRAM I/O (host provides these layouts directly) ----
    xT = nc.dram_tensor("xT", [128, n_et, s_loc], bf16, kind="ExternalInput").ap()
    yT = nc.dram_tensor("yT", [128, n_et, t_len], bf16, kind="ExternalInput").ap()
    wq2 = nc.dram_tensor("wq2", [128, n_hp, n_et, 128], bf16, kind="ExternalInput").ap()
    wk2 = nc.dram_tensor("wk2", [128, n_hp, n_et, 128], bf16, kind="ExternalInput").ap()
    wv = nc.dram_tensor("wv", [128, n_et, c_dim], bf16, kind="ExternalInput").ap()
    wo = nc.dram_tensor("wo", [128, n_hp, e_dim], bf16, kind="ExternalInput").ap()
    bq2 = nc.dram_tensor("bq2", [1, n_hp * 128], bf16, kind="ExternalInput").ap()
    bk2 = nc.dram_tensor("bk2", [1, n_hp * 128], bf16, kind="ExternalInput").ap()
    bvc = nc.dram_tensor("bvc", [1, c_dim], bf16, kind="ExternalInput").ap()
    bo_in = nc.dram_tensor("bo_in", [1, e_dim], f32, kind="ExternalInput").ap()
    out = nc.dram_tensor("out", [s_loc, e_dim], f32, kind="ExternalOutput").ap()

    from contextlib import ExitStack

    dbg = {}
    if debug_taps:
        dbg["v"] = nc.dram_tensor(
            "dbg_v", [128, t_len // 128, 128 * n_hp], dt.bfloat16, kind="ExternalOutput"
        ).ap()
        dbg["oT"] = nc.dram_tensor(
            "dbg_oT", [128, n_hp, s_loc], dt.bfloat16, kind="ExternalOutput"
        ).ap()
        dbg["qt0"] = nc.dram_tensor(
            "dbg_qt0", [128, s_loc], dt.bfloat16, kind="ExternalOutput"
        ).ap()
        dbg["kt0"] = nc.dram_tensor(
            "dbg_kt0", [128, t_len], dt.bfloat16, kind="ExternalOutput"
        ).ap()
        dbg["rcp0"] = nc.dram_tensor(
            "dbg_rcp0", [33, min(512, s_loc)], dt.float32, kind="ExternalOutput"
        ).ap()
        dbg["exp0"] = nc.dram_tensor(
            "dbg_exp0", [128, 2, min(512, s_loc)], dt.bfloat16, kind="ExternalOutput"
        ).ap()

    with tile.TileContext(nc) as tc, ExitStack() as ctx:
        consts = ctx.enter_context(tc.tile_pool(name="consts", bufs=1))
        scr_pool = ctx.enter_context(tc.tile_pool(name="scr", bufs=2, space="DRAM"))
        qt_pool = ctx.enter_context(tc.tile_pool(name="qt", bufs=2))
        kt_pool = ctx.enter_context(tc.tile_pool(name="kt", bufs=2))
        exp_pool = ctx.enter_context(tc.tile_pool(name="expp", bufs=3))
        rbc_pool = ctx.enter_context(tc.tile_pool(name="rbc", bufs=2))
        rcp_pool = ctx.enter_context(tc.tile_pool(name="rcp", bufs=2))
        osb_pool = ctx.enter_context(tc.tile_pool(name="osb", bufs=2))
        sc_ps = ctx.enter_context(tc.tile_pool(name="scps", bufs=2, space="PSUM"))
        acc_ps = ctx.enter_context(tc.tile_pool(name="accps", bufs=2, space="PSUM"))
        sum_ps = ctx.enter_context(tc.tile_pool(name="sumps", bufs=2, space="PSUM"))

        # ---- resident loads ----
        xT_sb = consts.tile([128, n_et, s_loc], bf16)
        nc.sync.dma_start(out=xT_sb, in_=xT)
        yT_sb = consts.tile([128, n_et, t_len], bf16)
        nc.sync.dma_start(out=yT_sb, in_=yT)
        wq_sb = consts.tile([128, n_hp, n_et, 128], bf16)
        nc.sync.dma_start(out=wq_sb, in_=wq2)
        wk_sb = consts.tile([128, n_hp, n_et, 128], bf16)
        nc.sync.dma_start(out=wk_sb, in_=wk2)
        wv_sb = consts.tile([128, n_et, c_dim], bf16)
        nc.sync.dma_start(out=wv_sb, in_=wv)
        bq_sb = consts.tile([1, n_hp * 128], bf16)
        nc.sync.dma_start(out=bq_sb, in_=bq2)
        bk_sb = consts.tile([1, n_hp * 128], bf16)
        nc.sync.dma_start(out=bk_sb, in_=bk2)
        bv_sb = consts.tile([1, c_dim], bf16)
        nc.sync.dma_start(out=bv_sb, in_=bvc)

        ones_col = consts.tile([128, 1], bf16)
        nc.vector.memset(ones_col, 1.0)
        ones_row = consts.tile([1, 512], bf16)
        nc.vector.memset(ones_row, 1.0)

        v_sb = consts.tile([128, n_tt, c_dim], bf16)
        oT_all = consts.tile([128, n_hp, s_loc], bf16)

        def v_proj_chunk(vc):
            # V[:, vc*vch : (vc+1)*vch] in natural [t, c] layout
            for tt in range(n_tt):
                ps = acc_ps.tile([128, vch], f32, tag="acc")
                for et in range(n_et):
                    nc.tensor.matmul(
                        out=ps,
                        lhsT=yT_sb[:, et, tt * 128 : (tt + 1) * 128],
                        rhs=wv_sb[:, et, vc * vch : (vc + 1) * vch],
                        start=(et == 0),
                        stop=False,
                    )
                nc.tensor.matmul(
                    out=ps,
                    lhsT=ones_row[0:1, 0:128],
                    rhs=bv_sb[0:1, vc * vch : (vc + 1) * vch],
                    start=False,
                    stop=True,
                )
                nc.vector.tensor_copy(
                    out=v_sb[:, tt, vc * vch : (vc + 1) * vch], in_=ps
                )

        def qk_proj(hp):
            qt = qt_pool.tile([128, s_loc], bf16, tag="qt")
            for sc in range(n_qc):
                ps = acc_ps.tile([128, qch], f32, tag="acc")
                for et in range(n_et):
                    nc.tensor.matmul(
                        out=ps,
                        lhsT=wq_sb[:, hp, et, :],
                        rhs=xT_sb[:, et, sc * qch : (sc + 1) * qch],
                        start=(et == 0),
                        stop=False,
                    )
                nc.tensor.matmul(
                    out=ps,
                    lhsT=bq_sb[0:1, hp * 128 : (hp + 1) * 128],
                    rhs=ones_row[0:1, 0:qch],
                    start=False,
                    stop=True,
                )
                nc.vector.tensor_copy(out=qt[:, sc * qch : (sc + 1) * qch], in_=ps)
            kt = kt_pool.tile([128, t_len], bf16, tag="kt")
            for tc_ in range(t_len // tch):
                ps = acc_ps.tile([128, tch], f32, tag="acc")
                for et in range(n_et):
                    nc.tensor.matmul(
                        out=ps,
                        lhsT=wk_sb[:, hp, et, :],
                        rhs=yT_sb[:, et, tc_ * tch : (tc_ + 1) * tch],
                        start=(et == 0),
                        stop=False,
                    )
                nc.tensor.matmul(
                    out=ps,
                    lhsT=bk_sb[0:1, hp * 128 : (hp + 1) * 128],
                    rhs=ones_row[0:1, 0:tch],
                    start=False,
                    stop=True,
                )
                nc.vector.tensor_copy(out=kt[:, tc_ * tch : (tc_ + 1) * tch], in_=ps)
            return qt, kt

        n_vc = c_dim // vch  # V column chunks
        vc_every = max(1, n_hp // n_vc) if n_vc else n_hp

        for hp in range(n_hp):
            if n_vc and hp % vc_every == 0 and hp // vc_every < n_vc:
                v_proj_chunk(hp // vc_every)
            qt, kt = qk_proj(hp)
            if debug_taps and hp == 0:
                nc.sync.dma_start(out=dbg["qt0"], in_=qt)
                nc.sync.dma_start(out=dbg["kt0"], in_=kt)

            for qc in range(n_qc):
                o_ps = acc_ps.tile([128, qch], f32, tag="acc")
                s_ps = sum_ps.tile([33, qch], f32, tag="sum")
                for tt in range(n_tt):
                    sc_tile = sc_ps.tile([128, 2, qch], f32, tag="sc")
                    # scoresT for head a (contraction rows 0:64) and head b
                    # (rows 64:128) — concurrent via PE row tiling.
                    nc.tensor.matmul(
                        out=sc_tile[:, 0, :],
                        lhsT=kt[0:64, tt * 128 : (tt + 1) * 128],
                        rhs=qt[0:64, qc * qch : (qc + 1) * qch],
                        start=True,
                        stop=True,
                    )
                    nc.tensor.matmul(
                        out=sc_tile[:, 1, :],
                        lhsT=kt[64:128, tt * 128 : (tt + 1) * 128],
                        rhs=qt[64:128, qc * qch : (qc + 1) * qch],
                        start=True,
                        stop=True,
                    )
                    exp_t = exp_pool.tile([128, 2, qch], bf16, tag="exp")
                    nc.scalar.activation(
                        out=exp_t,
                        in_=sc_tile,
                        func=_mybir().ActivationFunctionType.Exp,
                        scale=0.125,
                    )
                    if debug_taps and hp == 0 and qc == 0 and tt == 0:
                        nc.sync.dma_start(out=dbg["exp0"], in_=exp_t)
                    first, last = tt == 0, tt == n_tt - 1
                    # attnV: head a -> psum rows 0:64, head b -> rows 64:128
                    nc.tensor.matmul(
                        out=o_ps[0:64, :],
                        lhsT=v_sb[:, tt, hp * 128 : hp * 128 + 64],
                        rhs=exp_t[:, 0, :],
                        start=first,
                        stop=last,
                        skip_group_check=True,
                    )
                    nc.tensor.matmul(
                        out=o_ps[64:128, :],
                        lhsT=v_sb[:, tt, hp * 128 + 64 : hp * 128 + 128],
                        rhs=exp_t[:, 1, :],
                        start=first,
                        stop=last,
                        skip_group_check=True,
                    )
                    # softmax denominators (col groups 0 and 1)
                    nc.tensor.matmul(
                        out=s_ps[0:1, :],
                        lhsT=ones_col,
                        rhs=exp_t[:, 0, :],
                        start=first,
                        stop=last,
                        skip_group_check=True,
                    )
                    nc.tensor.matmul(
                        out=s_ps[32:33, :],
                        lhsT=ones_col,
                        rhs=exp_t[:, 1, :],
                        start=first,
                        stop=last,
                        skip_group_check=True,
                    )
                rcp = rcp_pool.tile([33, qch], f32, tag="rcp")
                nc.vector.reciprocal(out=rcp[0:1, :], in_=s_ps[0:1, :])
                nc.vector.reciprocal(out=rcp[32:33, :], in_=s_ps[32:33, :])
                if debug_taps and hp == 0 and qc == 0:
                    nc.sync.dma_start(out=dbg["rcp0"][0:1, :], in_=rcp[0:1, :])
                    nc.sync.dma_start(out=dbg["rcp0"][32:33, :], in_=rcp[32:33, :])
                # partition-broadcast 1/sum via a DRAM round trip (DMA can
                # broadcast a DRAM row across partitions; SBUF source can't)
                scr = scr_pool.tile([2, qch], f32, tag="scr")
                nc.sync.dma_start(out=scr[0:1, :], in_=rcp[0:1, :])
                nc.sync.dma_start(out=scr[1:2, :], in_=rcp[32:33, :])
                rbc = rbc_pool.tile([128, qch], f32, tag="rbc")
                nc.gpsimd.dma_start(
                    out=rbc[0:64, :], in_=scr[0:1, :].to_broadcast([64, qch])
                )
                nc.gpsimd.dma_start(
                    out=rbc[64:128, :], in_=scr[1:2, :].to_broadcast([64, qch])
                )
                nc.vector.tensor_mul(
                    oT_all[:, hp, qc * qch : (qc + 1) * qch], o_ps, rbc
                )

        if debug_taps:
            nc.sync.dma_start(out=dbg["v"], in_=v_sb)
            nc.sync.dma_start(out=dbg["oT"], in_=oT_all)

        # ---- output projection: out[s, e] = concatT.T @ Wo + bo ----
        wo_sb = consts.tile([128, n_hp, e_dim], bf16)
        nc.sync.dma_start(out=wo_sb, in_=wo)
        bo_bc = consts.tile([128, e_dim], f32)
        nc.gpsimd.dma_start(out=bo_bc, in_=bo_in[0:1, :].to_broadcast([128, e_dim]))

        for st in range(n_st):
            for ec in range(n_ec):
                ps = acc_ps.tile([128, ech], f32, tag="acc")
                for ct in range(n_hp):
                    nc.tensor.matmul(
                        out=ps,
                        lhsT=oT_all[:, ct, st * 128 : (st + 1) * 128],
                        rhs=wo_sb[:, ct, ec * ech : (ec + 1) * ech],
                        start=(ct == 0),
                        stop=(ct == n_hp - 1),
                    )
                o_sb = osb_pool.tile([128, ech], f32, tag="osb")
                nc.vector.tensor_add(
                    o_sb, ps, bo_bc[:, ec * ech : (ec + 1) * ech]
                )
                nc.sync.dma_start(
                    out=out[st * 128 : (st + 1) * 128, ec * ech : (ec + 1) * ech],
                    in_=o_sb,
                )

    nc.compile()
    return nc


def _mybir():
    from concourse import mybir

    return mybir


def _bf16(a):
    import ml_dtypes

    return np.ascontiguousarray(a).astype(ml_dtypes.bfloat16)


def host_prep_shared(Wq, bq, Wk, bk, Wv, bv, Wo, bo, n_hp=8, n_et=8):
    """Pack weights into the kernel's DRAM layouts (shared by all cores)."""
    e_dim = 128 * n_et

    def pack_pairs(W):
        # [H, E, D] -> [p, hp, et, m] with m = j*64+d, head = 2*hp+j
        n_heads = 2 * n_hp
        Wr = W[:n_heads].reshape(n_hp, 2, e_dim, D)  # hp, j, e, d
        arr = Wr.transpose(2, 0, 1, 3).reshape(e_dim, n_hp, 128)  # e, hp, m
        arr = arr.reshape(n_et, 128, n_hp, 128).transpose(1, 2, 0, 3)
        return np.ascontiguousarray(arr)  # [p, hp, et, m]

    c_dim = 128 * n_hp
    wv_cat = Wv[: 2 * n_hp].transpose(1, 0, 2).reshape(e_dim, c_dim)  # [e, c]
    wv_arr = wv_cat.reshape(n_et, 128, c_dim).transpose(1, 0, 2)  # [p, et, c]
    wo_arr = Wo[:c_dim].reshape(n_hp, 128, e_dim).transpose(1, 0, 2)  # [p, ct, e]

    return {
        "wq2": _bf16(pack_pairs(Wq)),
        "wk2": _bf16(pack_pairs(Wk)),
        "wv": _bf16(np.ascontiguousarray(wv_arr)),
        "wo": _bf16(np.ascontiguousarray(wo_arr)),
        "bq2": _bf16(bq[: 2 * n_hp].reshape(1, n_hp * 128)),
        "bk2": _bf16(bk[: 2 * n_hp].reshape(1, n_hp * 128)),
        "bvc": _bf16(bv[: 2 * n_hp].reshape(1, c_dim)),
        "bo_in": np.ascontiguousarray(bo.reshape(1, e_dim)).astype(np.float32),
    }


def host_prep_xt(mat, n_et=8):
    """[rows, E] -> [p, et, rows] transposed tiled layout, bf16."""
    rows, e_dim = mat.shape
    assert e_dim == 128 * n_et
    arr = mat.T.reshape(n_et, 128, rows).transpose(1, 0, 2)
    return _bf16(arr)


def kernel(x, y, Wq, bq, Wk, bk, Wv, bv, Wo, bo):
    import sys

    if "/opt/trn_rl_repo" not in sys.path:
        sys.path.insert(0, "/opt/trn_rl_repo")
    from concourse import bass_utils

    x = np.asarray(x, dtype=np.float32)
    y = np.asarray(y, dtype=np.float32)

    if "prog" not in _compiled:
        _compiled["prog"] = build_program()
    nc = _compiled["prog"]

    shared = host_prep_shared(
        np.asarray(Wq, np.float32),
        np.asarray(bq, np.float32),
        np.asarray(Wk, np.float32),
        np.asarray(bk, np.float32),
        np.asarray(Wv, np.float32),
        np.asarray(bv, np.float32),
        np.asarray(Wo, np.float32),
        np.asarray(bo, np.float32),
    )
    yT_b = [host_prep_xt(y[b]) for b in range(B)]
    in_maps = []
    for c in range(N_CORES):
        b, half = c // 2, c % 2
        m = dict(shared)
        m["xT"] = host_prep_xt(x[b, half * 1024 : (half + 1) * 1024, :])
        m["yT"] = yT_b[b]
        in_maps.append(m)

    import os

    trace = os.environ.get("TRN_ATTN_TRACE", "0") == "1"
    res = bass_utils.run_bass_kernel_spmd(
        nc, in_maps, core_ids=list(range(N_CORES)), trace=trace
    )
    _compiled["last_results"] = res
    out = np.empty((B, S, E), dtype=np.float32)
    for c in range(N_CORES):
        b, half = c // 2, c % 2
        out[b, half * 1024 : (half + 1) * 1024, :] = res.results[c]["out"]
    return out


# revision 24
# speedup vs baseline: 1.4487x; 1.4487x over previous
"""Cross multi-head attention on 8 Trainium2 NeuronCores.

Sharding: pure data-parallel. Core c handles batch b = c//2, query-half
h = c%2 (1024 of 2048 query rows). K/V are recomputed per seq-half (25%
extra PE work) which avoids all collectives.

Per-core kernel (all in T-major layout so no on-chip transposes at all;
the host pre-transposes x/y and pre-packs the weights):
  QT[hp]  [128,Sl] = Wq2[hp].T @ xT      (head-pair packed: rows 0:64 head a,
  KT[hp]  [128,T]  = Wk2[hp].T @ yT       rows 64:128 head b; bias fused into
                                          the PSUM->SBUF copy on DVE)
  V'      [t,h,65] = yT.T @ Wv_cat | 1   (natural layout + ones column)
  scoresT [t,q]    = KT_h.T @ QT_h       (K=64 contraction, two heads run
                                          concurrently via PE row tiling)
  expT    = exp(scoresT * 0.125)         (one ACT op per psum pair tile)
  oT'     += V'_h.T @ expT               (M=65: row 64 accumulates the softmax
                                          denominator for free)
  oT      = oT'[0:64] * bcast(1/oT'[64]) (deferred normalization, one DVE
                                          pass; partition-shifted for head b)
  out     = concatT.T @ Wo + bo
Matmul inputs bf16 (fp32 PSUM accumulation), softmax in fp32.
"""

import numpy as np

B, S, T, E, H, D = 4, 2048, 2048, 1024, 16, 64
N_CORES = 8

_compiled = {}


def _dt():
    from concourse import mybir

    return mybir.dt


def _mybir():
    from concourse import mybir

    return mybir


def build_program(n_hp=8, s_loc=1024, t_len=2048, n_et=8, debug_taps=False):
    """Emit the per-core bass program. Sizes parameterizable for sim tests.

    n_hp: head pairs (heads = 2*n_hp), s_loc: query rows on this core,
    t_len: key rows, n_et: contraction tiles (emb dim = 128*n_et).
    """
    import concourse.tile as tile
    from concourse import bacc

    dt = _dt()
    bf16 = dt.bfloat16
    f32 = dt.float32

    e_dim = 128 * n_et
    c_dim = 128 * n_hp  # concat dim on this core's heads
    n_h = 2 * n_hp
    n_tt = t_len // 128  # key tiles
    qch = min(512, s_loc)  # query chunk width
    tch = min(512, t_len)
    ech = min(512, e_dim)
    vch = min(512, c_dim)
    n_qc = s_loc // qch  # query chunks for attention
    n_st = s_loc // 128  # output row tiles
    n_ec = e_dim // ech  # output col chunks

    nc = bacc.Bacc("TRN2", target_bir_lowering=False, debug=False)

    # ---- DRAM I/O (host provides these layouts directly) ----
    xT = nc.dram_tensor("xT", [128, n_et, s_loc], bf16, kind="ExternalInput").ap()
    yT = nc.dram_tensor("yT", [128, n_et, t_len], bf16, kind="ExternalInput").ap()
    wq2 = nc.dram_tensor("wq2", [128, n_hp, n_et, 128], bf16, kind="ExternalInput").ap()
    wk2 = nc.dram_tensor("wk2", [128, n_hp, n_et, 128], bf16, kind="ExternalInput").ap()
    wv = nc.dram_tensor("wv", [128, n_et, c_dim], bf16, kind="ExternalInput").ap()
    wo = nc.dram_tensor("wo", [128, n_hp, e_dim], bf16, kind="ExternalInput").ap()
    bqc = nc.dram_tensor("bqc", [128, n_hp], f32, kind="ExternalInput").ap()
    bkc = nc.dram_tensor("bkc", [128, n_hp], f32, kind="ExternalInput").ap()
    bvc = nc.dram_tensor("bvc", [1, c_dim], bf16, kind="ExternalInput").ap()
    bo_in = nc.dram_tensor("bo_in", [1, e_dim], f32, kind="ExternalInput").ap()
    out = nc.dram_tensor("out", [s_loc, e_dim], f32, kind="ExternalOutput").ap()

    from contextlib import ExitStack

    dbg = {}
    if debug_taps:
        dbg["v"] = nc.dram_tensor(
            "dbg_v", [128, n_tt, n_h, 65], dt.bfloat16, kind="ExternalOutput"
        ).ap()
        dbg["oT"] = nc.dram_tensor(
            "dbg_oT", [128, n_hp, s_loc], dt.bfloat16, kind="ExternalOutput"
        ).ap()
        dbg["qt0"] = nc.dram_tensor(
            "dbg_qt0", [128, s_loc], dt.bfloat16, kind="ExternalOutput"
        ).ap()
        dbg["kt0"] = nc.dram_tensor(
            "dbg_kt0", [128, t_len], dt.bfloat16, kind="ExternalOutput"
        ).ap()
        dbg["rcp0"] = nc.dram_tensor(
            "dbg_rcp0", [33, qch], dt.float32, kind="ExternalOutput"
        ).ap()
        dbg["exp0"] = nc.dram_tensor(
            "dbg_exp0", [128, 2, qch], dt.bfloat16, kind="ExternalOutput"
        ).ap()

    with tile.TileContext(nc) as tc, ExitStack() as ctx:
        consts = ctx.enter_context(tc.tile_pool(name="consts", bufs=1))
        scr_pool = ctx.enter_context(tc.tile_pool(name="scr", bufs=2, space="DRAM"))
        qt_pool = ctx.enter_context(tc.tile_pool(name="qt", bufs=2))
        kt_pool = ctx.enter_context(tc.tile_pool(name="kt", bufs=2))
        exp_pool = ctx.enter_context(tc.tile_pool(name="expp", bufs=3))
        osc_pool = ctx.enter_context(tc.tile_pool(name="osc", bufs=4))
        rbc_pool = ctx.enter_context(tc.tile_pool(name="rbc", bufs=4))
        osb_pool = ctx.enter_context(tc.tile_pool(name="osb", bufs=2))
        sc_ps = ctx.enter_context(tc.tile_pool(name="scps", bufs=2, space="PSUM"))
        acc_ps = ctx.enter_context(tc.tile_pool(name="accps", bufs=2, space="PSUM"))
        o_ps_pool = ctx.enter_context(tc.tile_pool(name="ops", bufs=2, space="PSUM"))

        # ---- resident loads (big ones chunked per e-tile for early start) ----
        xT_sb = consts.tile([128, n_et, s_loc], bf16)
        wq_sb = consts.tile([128, n_hp, n_et, 128], bf16)
        yT_sb = consts.tile([128, n_et, t_len], bf16)
        wk_sb = consts.tile([128, n_hp, n_et, 128], bf16)
        wv_sb = consts.tile([128, n_et, c_dim], bf16)
        for et in range(n_et):
            nc.sync.dma_start(out=xT_sb[:, et, :], in_=xT[:, et, :])
        nc.sync.dma_start(out=wq_sb, in_=wq2)
        for et in range(n_et):
            nc.sync.dma_start(out=yT_sb[:, et, :], in_=yT[:, et, :])
        nc.sync.dma_start(out=wk_sb, in_=wk2)
        nc.sync.dma_start(out=wv_sb, in_=wv)
        bqc_sb = consts.tile([128, n_hp], f32)
        nc.sync.dma_start(out=bqc_sb, in_=bqc)
        bkc_sb = consts.tile([128, n_hp], f32)
        nc.sync.dma_start(out=bkc_sb, in_=bkc)
        bv_sb = consts.tile([1, c_dim], bf16)
        nc.sync.dma_start(out=bv_sb, in_=bvc)

        ones_row = consts.tile([1, 512], bf16)
        nc.vector.memset(ones_row, 1.0)

        # V' with a ones column per head: [p, tt, head, 65]
        v_sb = consts.tile([128, n_tt, n_h, 65], bf16)
        nc.vector.memset(v_sb[:, :, :, 64:65], 1.0)
        oT_all = consts.tile([128, n_hp, s_loc], bf16)

        def v_proj_chunk(vc):
            # V[:, vc*vch : (vc+1)*vch] in natural [t, c] layout
            nhc = vch // 64  # heads covered by this chunk
            h0 = vc * nhc
            for tt in range(n_tt):
                ps = acc_ps.tile([128, vch], f32, tag="acc")
                for et in range(n_et):
                    nc.tensor.matmul(
                        out=ps,
                        lhsT=yT_sb[:, et, tt * 128 : (tt + 1) * 128],
                        rhs=wv_sb[:, et, vc * vch : (vc + 1) * vch],
                        start=(et == 0),
                        stop=False,
                    )
                nc.tensor.matmul(
                    out=ps,
                    lhsT=ones_row[0:1, 0:128],
                    rhs=bv_sb[0:1, vc * vch : (vc + 1) * vch],
                    start=False,
                    stop=True,
                )
                nc.vector.tensor_copy(
                    out=v_sb[:, tt, h0 : h0 + nhc, 0:64],
                    in_=ps.rearrange("p (h d) -> p h d", d=64),
                )

        def qk_proj(hp):
            qt = qt_pool.tile([128, s_loc], bf16, tag="qt")
            for sc in range(n_qc):
                ps = acc_ps.tile([128, qch], f32, tag="acc")
                for et in range(n_et):
                    nc.tensor.matmul(
                        out=ps,
                        lhsT=wq_sb[:, hp, et, :],
                        rhs=xT_sb[:, et, sc * qch : (sc + 1) * qch],
                        start=(et == 0),
                        stop=(et == n_et - 1),
                    )
                nc.vector.tensor_scalar_add(
                    out=qt[:, sc * qch : (sc + 1) * qch],
                    in0=ps,
                    scalar1=bqc_sb[:, hp : hp + 1],
                )
            kt = kt_pool.tile([128, t_len], bf16, tag="kt")
            for tc_ in range(t_len // tch):
                ps = acc_ps.tile([128, tch], f32, tag="acc")
                for et in range(n_et):
                    nc.tensor.matmul(
                        out=ps,
                        lhsT=wk_sb[:, hp, et, :],
                        rhs=yT_sb[:, et, tc_ * tch : (tc_ + 1) * tch],
                        start=(et == 0),
                        stop=(et == n_et - 1),
                    )
                nc.vector.tensor_scalar_add(
                    out=kt[:, tc_ * tch : (tc_ + 1) * tch],
                    in0=ps,
                    scalar1=bkc_sb[:, hp : hp + 1],
                )
            return qt, kt

        n_vc = c_dim // vch  # V column chunks
        vc_every = max(1, n_hp // n_vc) if n_vc else n_hp

        for hp in range(n_hp):
            qt, kt = qk_proj(hp)
            if n_vc and hp % vc_every == 0 and hp // vc_every < n_vc:
                v_proj_chunk(hp // vc_every)
            if debug_taps and hp == 0:
                nc.sync.dma_start(out=dbg["qt0"], in_=qt)
                nc.sync.dma_start(out=dbg["kt0"], in_=kt)

            for qc in range(n_qc):
                o_a = o_ps_pool.tile([65, qch], f32, tag="o")
                o_b = o_ps_pool.tile([65, qch], f32, tag="o")
                for tt in range(n_tt):
                    sc_tile = sc_ps.tile([128, 2, qch], f32, tag="sc")
                    # scoresT for head a (contraction rows 0:64) and head b
                    # (rows 64:128) — concurrent via PE row tiling.
                    nc.tensor.matmul(
                        out=sc_tile[:, 0, :],
                        lhsT=kt[0:64, tt * 128 : (tt + 1) * 128],
                        rhs=qt[0:64, qc * qch : (qc + 1) * qch],
                        start=True,
                        stop=True,
                    )
                    nc.tensor.matmul(
                        out=sc_tile[:, 1, :],
                        lhsT=kt[64:128, tt * 128 : (tt + 1) * 128],
                        rhs=qt[64:128, qc * qch : (qc + 1) * qch],
                        start=True,
                        stop=True,
                    )
                    exp_t = exp_pool.tile([128, 2, qch], bf16, tag="exp")
                    nc.scalar.activation(
                        out=exp_t,
                        in_=sc_tile,
                        func=_mybir().ActivationFunctionType.Exp,
                        scale=0.125,
                    )
                    if debug_taps and hp == 0 and qc == 0 and tt == 0:
                        nc.sync.dma_start(out=dbg["exp0"], in_=exp_t)
                    first, last = tt == 0, tt == n_tt - 1
                    # attnV with ones column: row 64 = softmax denominator
                    nc.tensor.matmul(
                        out=o_a,
                        lhsT=v_sb[:, tt, 2 * hp, :],
                        rhs=exp_t[:, 0, :],
                        start=first,
                        stop=last,
                    )
                    nc.tensor.matmul(
                        out=o_b,
                        lhsT=v_sb[:, tt, 2 * hp + 1, :],
                        rhs=exp_t[:, 1, :],
                        start=first,
                        stop=last,
                    )
                # fast psum evacuation, then normalize SBUF-side
                osc_a = osc_pool.tile([65, qch], f32, tag="osc")
                nc.vector.tensor_copy(out=osc_a, in_=o_a)
                osc_b = osc_pool.tile([65, qch], f32, tag="osc")
                nc.vector.tensor_copy(out=osc_b, in_=o_b)
                nc.vector.reciprocal(out=osc_a[64:65, :], in_=osc_a[64:65, :])
                nc.vector.reciprocal(out=osc_b[64:65, :], in_=osc_b[64:65, :])
                if debug_taps and hp == 0 and qc == 0:
                    nc.sync.dma_start(out=dbg["rcp0"][0:1, :], in_=osc_a[64:65, :])
                    nc.sync.dma_start(out=dbg["rcp0"][32:33, :], in_=osc_b[64:65, :])
                # partition-broadcast 1/sum via a DRAM round trip
                scr = scr_pool.tile([2, qch], f32, tag="scr")
                nc.sync.dma_start(out=scr[0:1, :], in_=osc_a[64:65, :])
                nc.sync.dma_start(out=scr[1:2, :], in_=osc_b[64:65, :])
                rbc_a = rbc_pool.tile([64, qch], f32, tag="rbc")
                nc.gpsimd.dma_start(
                    out=rbc_a, in_=scr[0:1, :].to_broadcast([64, qch])
                )
                rbc_b = rbc_pool.tile([64, qch], f32, tag="rbc")
                nc.gpsimd.dma_start(
                    out=rbc_b, in_=scr[1:2, :].to_broadcast([64, qch])
                )
                nc.vector.tensor_mul(
                    oT_all[0:64, hp, qc * qch : (qc + 1) * qch],
                    osc_a[0:64, :],
                    rbc_a,
                )
                nc.vector.tensor_mul(
                    oT_all[64:128, hp, qc * qch : (qc + 1) * qch],
                    osc_b[0:64, :],
                    rbc_b,
                )

        if debug_taps:
            nc.sync.dma_start(out=dbg["v"], in_=v_sb)
            nc.sync.dma_start(out=dbg["oT"], in_=oT_all)

        # ---- output projection: out[s, e] = concatT.T @ Wo + bo ----
        wo_sb = consts.tile([128, n_hp, e_dim], bf16)
        nc.sync.dma_start(out=wo_sb, in_=wo)
        bo_bc = consts.tile([128, e_dim], f32)
        nc.gpsimd.dma_start(out=bo_bc, in_=bo_in[0:1, :].to_broadcast([128, e_dim]))

        for st in range(n_st):
            for ec in range(n_ec):
                ps = acc_ps.tile([128, ech], f32, tag="acc")
                for ct in range(n_hp):
                    nc.tensor.matmul(
                        out=ps,
                        lhsT=oT_all[:, ct, st * 128 : (st + 1) * 128],
                        rhs=wo_sb[:, ct, ec * ech : (ec + 1) * ech],
                        start=(ct == 0),
                        stop=(ct == n_hp - 1),
                    )
                o_sb = osb_pool.tile([128, ech], f32, tag="osb")
                nc.vector.tensor_add(o_sb, ps, bo_bc[:, ec * ech : (ec + 1) * ech])
                nc.sync.dma_start(
                    out=out[st * 128 : (st + 1) * 128, ec * ech : (ec + 1) * ech],
                    in_=o_sb,
                )

    nc.compile()
    return nc


def _bf16(a):
    import ml_dtypes

    return np.ascontiguousarray(a).astype(ml_dtypes.bfloat16)


def host_prep_shared(Wq, bq, Wk, bk, Wv, bv, Wo, bo, n_hp=8, n_et=8):
    """Pack weights into the kernel's DRAM layouts (shared by all cores)."""
    e_dim = 128 * n_et

    def pack_pairs(W):
        # [H, E, D] -> [p, hp, et, m] with m = j*64+d, head = 2*hp+j
        n_heads = 2 * n_hp
        Wr = W[:n_heads].reshape(n_hp, 2, e_dim, D)  # hp, j, e, d
        arr = Wr.transpose(2, 0, 1, 3).reshape(e_dim, n_hp, 128)  # e, hp, m
        arr = arr.reshape(n_et, 128, n_hp, 128).transpose(1, 2, 0, 3)
        return np.ascontiguousarray(arr)  # [p, hp, et, m]

    def bias_cols(b):
        # [H, D] -> [p, hp] with p = j*64+d
        return np.ascontiguousarray(
            b[: 2 * n_hp].reshape(n_hp, 2, 64).transpose(1, 2, 0).reshape(128, n_hp)
        ).astype(np.float32)

    c_dim = 128 * n_hp
    wv_cat = Wv[: 2 * n_hp].transpose(1, 0, 2).reshape(e_dim, c_dim)  # [e, c]
    wv_arr = wv_cat.reshape(n_et, 128, c_dim).transpose(1, 0, 2)  # [p, et, c]
    wo_arr = Wo[:c_dim].reshape(n_hp, 128, e_dim).transpose(1, 0, 2)  # [p, ct, e]

    return {
        "wq2": _bf16(pack_pairs(Wq)),
        "wk2": _bf16(pack_pairs(Wk)),
        "wv": _bf16(np.ascontiguousarray(wv_arr)),
        "wo": _bf16(np.ascontiguousarray(wo_arr)),
        "bqc": bias_cols(bq),
        "bkc": bias_cols(bk),
        "bvc": _bf16(bv[: 2 * n_hp].reshape(1, c_dim)),
        "bo_in": np.ascontiguousarray(bo.reshape(1, e_dim)).astype(np.float32),
    }


def host_prep_xt(mat, n_et=8):
    """[rows, E] -> [p, et, rows] transposed tiled layout, bf16."""
    rows, e_dim = mat.shape
    assert e_dim == 128 * n_et
    arr = mat.T.reshape(n_et, 128, rows).transpose(1, 0, 2)
    return _bf16(arr)


def kernel(x, y, Wq, bq, Wk, bk, Wv, bv, Wo, bo):
    import os
    import sys

    if "/opt/trn_rl_repo" not in sys.path:
        sys.path.insert(0, "/opt/trn_rl_repo")
    from concourse import bass_utils

    x = np.asarray(x, dtype=np.float32)
    y = np.asarray(y, dtype=np.float32)

    if "prog" not in _compiled:
        _compiled["prog"] = build_program()
    nc = _compiled["prog"]

    shared = host_prep_shared(
        np.asarray(Wq, np.float32),
        np.asarray(bq, np.float32),
        np.asarray(Wk, np.float32),
        np.asarray(bk, np.float32),
        np.asarray(Wv, np.float32),
        np.asarray(bv, np.float32),
        np.asarray(Wo, np.float32),
        np.asarray(bo, np.float32),
    )
    yT_b = [host_prep_xt(y[b]) for b in range(B)]
    in_maps = []
    for c in range(N_CORES):
        b, half = c // 2, c % 2
        m = dict(shared)
        m["xT"] = host_prep_xt(x[b, half * 1024 : (half + 1) * 1024, :])
        m["yT"] = yT_b[b]
        in_maps.append(m)

    trace = os.environ.get("TRN_ATTN_TRACE", "0") == "1"
    res = bass_utils.run_bass_kernel_spmd(
        nc, in_maps, core_ids=list(range(N_CORES)), trace=trace
    )
    _compiled["last_results"] = res
    out = np.empty((B, S, E), dtype=np.float32)
    for c in range(N_CORES):
        b, half = c // 2, c % 2
        out[b, half * 1024 : (half + 1) * 1024, :] = res.results[c]["out"]
    return out


# revision 26
# speedup vs baseline: 1.5793x; 1.0902x over previous
"""Cross multi-head attention on 8 Trainium2 NeuronCores.

Sharding: pure data-parallel. Core c handles batch b = c//2, query-half
h = c%2 (1024 of 2048 query rows). K/V are recomputed per seq-half (25%
extra PE work) which avoids all collectives.

Per-core kernel (all in T-major layout so no on-chip transposes at all;
the host pre-transposes x/y and pre-packs the weights):
  QT[hp]  [128,Sl] = Wq2[hp].T @ xT      (head-pair packed: rows 0:64 head a,
  KT[hp]  [128,T]  = Wk2[hp].T @ yT       rows 64:128 head b; bias fused into
                                          the PSUM->SBUF copy on DVE)
  V'      [t,h,65] = yT.T @ Wv_cat | 1   (natural layout + ones column)
  scoresT [t,q]    = KT_h.T @ QT_h       (K=64 contraction, two heads run
                                          concurrently via PE row tiling)
  expT    = exp(scoresT * 0.125)         (one ACT op per psum pair tile)
  oT'     += V'_h.T @ expT               (M=65: row 64 accumulates the softmax
                                          denominator for free)
  oT      = oT'[0:64] * bcast(1/oT'[64]) (deferred normalization, one DVE
                                          pass; partition-shifted for head b)
  out     = concatT.T @ Wo + bo
Matmul inputs bf16 (fp32 PSUM accumulation), softmax in fp32.
"""

import numpy as np

B, S, T, E, H, D = 4, 2048, 2048, 1024, 16, 64
N_CORES = 8

_compiled = {}


def _dt():
    from concourse import mybir

    return mybir.dt


def _mybir():
    from concourse import mybir

    return mybir


def build_program(n_hp=8, s_loc=1024, t_len=2048, n_et=8, debug_taps=False):
    """Emit the per-core bass program. Sizes parameterizable for sim tests.

    n_hp: head pairs (heads = 2*n_hp), s_loc: query rows on this core,
    t_len: key rows, n_et: contraction tiles (emb dim = 128*n_et).
    """
    import concourse.tile as tile
    from concourse import bacc

    dt = _dt()
    bf16 = dt.bfloat16
    f32 = dt.float32

    e_dim = 128 * n_et
    c_dim = 128 * n_hp  # concat dim on this core's heads
    n_h = 2 * n_hp
    n_tt = t_len // 128  # key tiles
    qch = min(512, s_loc)  # query chunk width
    tch = min(512, t_len)
    ech = min(512, e_dim)
    vch = min(512, c_dim)
    n_qc = s_loc // qch  # query chunks for attention
    n_st = s_loc // 128  # output row tiles
    n_ec = e_dim // ech  # output col chunks

    nc = bacc.Bacc("TRN2", target_bir_lowering=False, debug=False)

    # ---- DRAM I/O (host provides these layouts directly) ----
    xT = nc.dram_tensor("xT", [128, n_et, s_loc], bf16, kind="ExternalInput").ap()
    yT = nc.dram_tensor("yT", [128, n_et, t_len], bf16, kind="ExternalInput").ap()
    wq2 = nc.dram_tensor("wq2", [128, n_hp, n_et, 128], bf16, kind="ExternalInput").ap()
    wk2 = nc.dram_tensor("wk2", [128, n_hp, n_et, 128], bf16, kind="ExternalInput").ap()
    wv = nc.dram_tensor("wv", [128, n_et, c_dim], bf16, kind="ExternalInput").ap()
    wo = nc.dram_tensor("wo", [128, n_hp, e_dim], bf16, kind="ExternalInput").ap()
    bqc = nc.dram_tensor("bqc", [128, n_hp], f32, kind="ExternalInput").ap()
    bkc = nc.dram_tensor("bkc", [128, n_hp], f32, kind="ExternalInput").ap()
    bvc = nc.dram_tensor("bvc", [1, c_dim], bf16, kind="ExternalInput").ap()
    bo_in = nc.dram_tensor("bo_in", [1, e_dim], f32, kind="ExternalInput").ap()
    out = nc.dram_tensor("out", [s_loc, e_dim], f32, kind="ExternalOutput").ap()

    from contextlib import ExitStack

    dbg = {}
    if debug_taps:
        dbg["v"] = nc.dram_tensor(
            "dbg_v", [128, n_tt, n_h, 65], dt.bfloat16, kind="ExternalOutput"
        ).ap()
        dbg["oT"] = nc.dram_tensor(
            "dbg_oT", [128, n_hp, s_loc], dt.bfloat16, kind="ExternalOutput"
        ).ap()
        dbg["qt0"] = nc.dram_tensor(
            "dbg_qt0", [128, s_loc], dt.bfloat16, kind="ExternalOutput"
        ).ap()
        dbg["kt0"] = nc.dram_tensor(
            "dbg_kt0", [128, t_len], dt.bfloat16, kind="ExternalOutput"
        ).ap()
        dbg["rcp0"] = nc.dram_tensor(
            "dbg_rcp0", [33, qch], dt.float32, kind="ExternalOutput"
        ).ap()
        dbg["exp0"] = nc.dram_tensor(
            "dbg_exp0", [128, 2, qch], dt.bfloat16, kind="ExternalOutput"
        ).ap()

    with tile.TileContext(nc) as tc, ExitStack() as ctx:
        consts = ctx.enter_context(tc.tile_pool(name="consts", bufs=1))
        scr_pool = ctx.enter_context(tc.tile_pool(name="scr", bufs=2, space="DRAM"))
        qt_pool = ctx.enter_context(tc.tile_pool(name="qt", bufs=2))
        kt_pool = ctx.enter_context(tc.tile_pool(name="kt", bufs=2))
        exp_pool = ctx.enter_context(tc.tile_pool(name="expp", bufs=3))
        osc_pool = ctx.enter_context(tc.tile_pool(name="osc", bufs=4))
        rbc_pool = ctx.enter_context(tc.tile_pool(name="rbc", bufs=4))
        osb_pool = ctx.enter_context(tc.tile_pool(name="osb", bufs=2))
        sc_ps = ctx.enter_context(tc.tile_pool(name="scps", bufs=2, space="PSUM"))
        acc_ps = ctx.enter_context(tc.tile_pool(name="accps", bufs=2, space="PSUM"))
        o_ps_pool = ctx.enter_context(tc.tile_pool(name="ops", bufs=2, space="PSUM"))

        # ---- resident loads (big ones chunked per e-tile for early start) ----
        xT_sb = consts.tile([128, n_et, s_loc], bf16)
        wq_sb = consts.tile([128, n_hp, n_et, 128], bf16)
        yT_sb = consts.tile([128, n_et, t_len], bf16)
        wk_sb = consts.tile([128, n_hp, n_et, 128], bf16)
        wv_sb = consts.tile([128, n_et, c_dim], bf16)
        for et in range(n_et):
            nc.sync.dma_start(out=xT_sb[:, et, :], in_=xT[:, et, :])
        nc.sync.dma_start(out=wq_sb[:, 0, :, :], in_=wq2[:, 0, :, :])
        for et in range(n_et):
            nc.sync.dma_start(out=yT_sb[:, et, :], in_=yT[:, et, :])
        nc.sync.dma_start(out=wk_sb[:, 0, :, :], in_=wk2[:, 0, :, :])
        nc.sync.dma_start(out=wv_sb[:, :, 0:vch], in_=wv[:, :, 0:vch])
        if n_hp > 1:
            nc.sync.dma_start(out=wq_sb[:, 1:, :, :], in_=wq2[:, 1:, :, :])
            nc.sync.dma_start(out=wk_sb[:, 1:, :, :], in_=wk2[:, 1:, :, :])
        if c_dim > vch:
            nc.sync.dma_start(out=wv_sb[:, :, vch:], in_=wv[:, :, vch:])
        bqc_sb = consts.tile([128, n_hp], f32)
        nc.sync.dma_start(out=bqc_sb, in_=bqc)
        bkc_sb = consts.tile([128, n_hp], f32)
        nc.sync.dma_start(out=bkc_sb, in_=bkc)
        bv_sb = consts.tile([1, c_dim], bf16)
        nc.sync.dma_start(out=bv_sb, in_=bvc)

        ones_row = consts.tile([1, 512], bf16)
        nc.vector.memset(ones_row, 1.0)

        # V' with a ones column per head: [p, tt, head, 65]
        v_sb = consts.tile([128, n_tt, n_h, 65], bf16)
        nc.vector.memset(v_sb[:, :, :, 64:65], 1.0)
        oT_all = consts.tile([128, n_hp, s_loc], bf16)

        def v_proj_chunk(vc):
            # V[:, vc*vch : (vc+1)*vch] in natural [t, c] layout
            nhc = vch // 64  # heads covered by this chunk
            h0 = vc * nhc
            for tt in range(n_tt):
                ps = acc_ps.tile([128, vch], f32, tag="acc")
                for et in range(n_et):
                    nc.tensor.matmul(
                        out=ps,
                        lhsT=yT_sb[:, et, tt * 128 : (tt + 1) * 128],
                        rhs=wv_sb[:, et, vc * vch : (vc + 1) * vch],
                        start=(et == 0),
                        stop=False,
                    )
                nc.tensor.matmul(
                    out=ps,
                    lhsT=ones_row[0:1, 0:128],
                    rhs=bv_sb[0:1, vc * vch : (vc + 1) * vch],
                    start=False,
                    stop=True,
                )
                nc.vector.tensor_copy(
                    out=v_sb[:, tt, h0 : h0 + nhc, 0:64],
                    in_=ps.rearrange("p (h d) -> p h d", d=64),
                )

        def qk_proj(hp):
            qt = qt_pool.tile([128, s_loc], bf16, tag="qt")
            for sc in range(n_qc):
                ps = acc_ps.tile([128, qch], f32, tag="acc")
                for et in range(n_et):
                    nc.tensor.matmul(
                        out=ps,
                        lhsT=wq_sb[:, hp, et, :],
                        rhs=xT_sb[:, et, sc * qch : (sc + 1) * qch],
                        start=(et == 0),
                        stop=(et == n_et - 1),
                    )
                nc.vector.tensor_scalar_add(
                    out=qt[:, sc * qch : (sc + 1) * qch],
                    in0=ps,
                    scalar1=bqc_sb[:, hp : hp + 1],
                )
            kt = kt_pool.tile([128, t_len], bf16, tag="kt")
            for tc_ in range(t_len // tch):
                ps = acc_ps.tile([128, tch], f32, tag="acc")
                for et in range(n_et):
                    nc.tensor.matmul(
                        out=ps,
                        lhsT=wk_sb[:, hp, et, :],
                        rhs=yT_sb[:, et, tc_ * tch : (tc_ + 1) * tch],
                        start=(et == 0),
                        stop=(et == n_et - 1),
                    )
                nc.vector.tensor_scalar_add(
                    out=kt[:, tc_ * tch : (tc_ + 1) * tch],
                    in0=ps,
                    scalar1=bkc_sb[:, hp : hp + 1],
                )
            return qt, kt

        n_vc = c_dim // vch  # V column chunks
        vc_every = max(1, n_hp // n_vc) if n_vc else n_hp

        for hp in range(n_hp):
            qt, kt = qk_proj(hp)
            if n_vc and hp % vc_every == 0 and hp // vc_every < n_vc:
                v_proj_chunk(hp // vc_every)
            if debug_taps and hp == 0:
                nc.sync.dma_start(out=dbg["qt0"], in_=qt)
                nc.sync.dma_start(out=dbg["kt0"], in_=kt)

            for qc in range(n_qc):
                o_a = o_ps_pool.tile([65, qch], f32, tag="o")
                o_b = o_ps_pool.tile([65, qch], f32, tag="o")
                for tt in range(n_tt):
                    sc_tile = sc_ps.tile([128, 2, qch], f32, tag="sc")
                    # scoresT for head a (contraction rows 0:64) and head b
                    # (rows 64:128) — concurrent via PE row tiling.
                    nc.tensor.matmul(
                        out=sc_tile[:, 0, :],
                        lhsT=kt[0:64, tt * 128 : (tt + 1) * 128],
                        rhs=qt[0:64, qc * qch : (qc + 1) * qch],
                        start=True,
                        stop=True,
                    )
                    nc.tensor.matmul(
                        out=sc_tile[:, 1, :],
                        lhsT=kt[64:128, tt * 128 : (tt + 1) * 128],
                        rhs=qt[64:128, qc * qch : (qc + 1) * qch],
                        start=True,
                        stop=True,
                    )
                    exp_t = exp_pool.tile([128, 2, qch], bf16, tag="exp")
                    nc.scalar.activation(
                        out=exp_t,
                        in_=sc_tile,
                        func=_mybir().ActivationFunctionType.Exp,
                        scale=0.125,
                    )
                    if debug_taps and hp == 0 and qc == 0 and tt == 0:
                        nc.sync.dma_start(out=dbg["exp0"], in_=exp_t)
                    first, last = tt == 0, tt == n_tt - 1
                    # attnV with ones column: row 64 = softmax denominator
                    nc.tensor.matmul(
                        out=o_a,
                        lhsT=v_sb[:, tt, 2 * hp, :],
                        rhs=exp_t[:, 0, :],
                        start=first,
                        stop=last,
                    )
                    nc.tensor.matmul(
                        out=o_b,
                        lhsT=v_sb[:, tt, 2 * hp + 1, :],
                        rhs=exp_t[:, 1, :],
                        start=first,
                        stop=last,
                    )
                # fast psum evacuation on ACT (keeps it off the DVE queue,
                # which is busy with reciprocals), then normalize SBUF-side
                osc_a = osc_pool.tile([65, qch], f32, tag="osc")
                nc.scalar.copy(out=osc_a, in_=o_a)
                osc_b = osc_pool.tile([65, qch], f32, tag="osc")
                nc.scalar.copy(out=osc_b, in_=o_b)
                nc.vector.reciprocal(out=osc_a[64:65, :], in_=osc_a[64:65, :])
                nc.vector.reciprocal(out=osc_b[64:65, :], in_=osc_b[64:65, :])
                if debug_taps and hp == 0 and qc == 0:
                    nc.sync.dma_start(out=dbg["rcp0"][0:1, :], in_=osc_a[64:65, :])
                    nc.sync.dma_start(out=dbg["rcp0"][32:33, :], in_=osc_b[64:65, :])
                # partition-broadcast 1/sum via a DRAM round trip
                scr = scr_pool.tile([2, qch], f32, tag="scr")
                nc.sync.dma_start(out=scr[0:1, :], in_=osc_a[64:65, :])
                nc.sync.dma_start(out=scr[1:2, :], in_=osc_b[64:65, :])
                rbc_a = rbc_pool.tile([64, qch], f32, tag="rbc")
                nc.gpsimd.dma_start(
                    out=rbc_a, in_=scr[0:1, :].to_broadcast([64, qch])
                )
                rbc_b = rbc_pool.tile([64, qch], f32, tag="rbc")
                nc.gpsimd.dma_start(
                    out=rbc_b, in_=scr[1:2, :].to_broadcast([64, qch])
                )
                nc.vector.tensor_mul(
                    oT_all[0:64, hp, qc * qch : (qc + 1) * qch],
                    osc_a[0:64, :],
                    rbc_a,
                )
                nc.vector.tensor_mul(
                    oT_all[64:128, hp, qc * qch : (qc + 1) * qch],
                    osc_b[0:64, :],
                    rbc_b,
                )

        if debug_taps:
            nc.sync.dma_start(out=dbg["v"], in_=v_sb)
            nc.sync.dma_start(out=dbg["oT"], in_=oT_all)

        # ---- output projection: out[s, e] = concatT.T @ Wo + bo ----
        wo_sb = consts.tile([128, n_hp, e_dim], bf16)
        nc.sync.dma_start(out=wo_sb, in_=wo)
        bo_bc = consts.tile([128, e_dim], f32)
        nc.gpsimd.dma_start(out=bo_bc, in_=bo_in[0:1, :].to_broadcast([128, e_dim]))

        for st in range(n_st):
            for ec in range(n_ec):
                ps = acc_ps.tile([128, ech], f32, tag="acc")
                for ct in range(n_hp):
                    nc.tensor.matmul(
                        out=ps,
                        lhsT=oT_all[:, ct, st * 128 : (st + 1) * 128],
                        rhs=wo_sb[:, ct, ec * ech : (ec + 1) * ech],
                        start=(ct == 0),
                        stop=(ct == n_hp - 1),
                    )
                o_sb = osb_pool.tile([128, ech], f32, tag="osb")
                nc.vector.tensor_add(o_sb, ps, bo_bc[:, ec * ech : (ec + 1) * ech])
                nc.sync.dma_start(
                    out=out[st * 128 : (st + 1) * 128, ec * ech : (ec + 1) * ech],
                    in_=o_sb,
                )

    nc.compile()
    return nc


def _bf16(a):
    import ml_dtypes

    return np.ascontiguousarray(a).astype(ml_dtypes.bfloat16)


def host_prep_shared(Wq, bq, Wk, bk, Wv, bv, Wo, bo, n_hp=8, n_et=8):
    """Pack weights into the kernel's DRAM layouts (shared by all cores)."""
    e_dim = 128 * n_et

    def pack_pairs(W):
        # [H, E, D] -> [p, hp, et, m] with m = j*64+d, head = 2*hp+j
        n_heads = 2 * n_hp
        Wr = W[:n_heads].reshape(n_hp, 2, e_dim, D)  # hp, j, e, d
        arr = Wr.transpose(2, 0, 1, 3).reshape(e_dim, n_hp, 128)  # e, hp, m
        arr = arr.reshape(n_et, 128, n_hp, 128).transpose(1, 2, 0, 3)
        return np.ascontiguousarray(arr)  # [p, hp, et, m]

    def bias_cols(b):
        # [H, D] -> [p, hp] with p = j*64+d
        return np.ascontiguousarray(
            b[: 2 * n_hp].reshape(n_hp, 2, 64).transpose(1, 2, 0).reshape(128, n_hp)
        ).astype(np.float32)

    c_dim = 128 * n_hp
    wv_cat = Wv[: 2 * n_hp].transpose(1, 0, 2).reshape(e_dim, c_dim)  # [e, c]
    wv_arr = wv_cat.reshape(n_et, 128, c_dim).transpose(1, 0, 2)  # [p, et, c]
    wo_arr = Wo[:c_dim].reshape(n_hp, 128, e_dim).transpose(1, 0, 2)  # [p, ct, e]

    return {
        "wq2": _bf16(pack_pairs(Wq)),
        "wk2": _bf16(pack_pairs(Wk)),
        "wv": _bf16(np.ascontiguousarray(wv_arr)),
        "wo": _bf16(np.ascontiguousarray(wo_arr)),
        "bqc": bias_cols(bq),
        "bkc": bias_cols(bk),
        "bvc": _bf16(bv[: 2 * n_hp].reshape(1, c_dim)),
        "bo_in": np.ascontiguousarray(bo.reshape(1, e_dim)).astype(np.float32),
    }


def host_prep_xt(mat, n_et=8):
    """[rows, E] -> [p, et, rows] transposed tiled layout, bf16."""
    rows, e_dim = mat.shape
    assert e_dim == 128 * n_et
    arr = mat.T.reshape(n_et, 128, rows).transpose(1, 0, 2)
    return _bf16(arr)


def kernel(x, y, Wq, bq, Wk, bk, Wv, bv, Wo, bo):
    import os
    import sys

    if "/opt/trn_rl_repo" not in sys.path:
        sys.path.insert(0, "/opt/trn_rl_repo")
    from concourse import bass_utils

    x = np.asarray(x, dtype=np.float32)
    y = np.asarray(y, dtype=np.float32)

    if "prog" not in _compiled:
        _compiled["prog"] = build_program()
    nc = _compiled["prog"]

    shared = host_prep_shared(
        np.asarray(Wq, np.float32),
        np.asarray(bq, np.float32),
        np.asarray(Wk, np.float32),
        np.asarray(bk, np.float32),
        np.asarray(Wv, np.float32),
        np.asarray(bv, np.float32),
        np.asarray(Wo, np.float32),
        np.asarray(bo, np.float32),
    )
    yT_b = [host_prep_xt(y[b]) for b in range(B)]
    in_maps = []
    for c in range(N_CORES):
        b, half = c // 2, c % 2
        m = dict(shared)
        m["xT"] = host_prep_xt(x[b, half * 1024 : (half + 1) * 1024, :])
        m["yT"] = yT_b[b]
        in_maps.append(m)

    trace = os.environ.get("TRN_ATTN_TRACE", "0") == "1"
    res = bass_utils.run_bass_kernel_spmd(
        nc, in_maps, core_ids=list(range(N_CORES)), trace=trace
    )
    _compiled["last_results"] = res
    out = np.empty((B, S, E), dtype=np.float32)
    for c in range(N_CORES):
        b, half = c // 2, c % 2
        out[b, half * 1024 : (half + 1) * 1024, :] = res.results[c]["out"]
    return out
